# revision 8
# baseline (speedup 1.0000x reference)
"""Trainium2 Bass kernel for single-head causal attention.

Problem: nn_Attention (dense_transformer): B=8, T=2048, C=1024, D=64, fp32.
    q = x @ Wq; k = x @ Wk; v = x @ Wv
    out = softmax(causal(q k^T / sqrt(C))) @ v

Sharding: data-parallel over batch - one batch element per NeuronCore (8 cores).
Weights replicated. The host marshals inputs into the on-chip layout (fp16,
x pre-transposed to [C, T] and blocked per DMA tile); each core runs an
identical program on its batch element.

Per-core algorithm (fp16 operands, fp32 PSUM accumulation):
  1. x^T arrives via DMA as xn[128, c8, 512] blocks (contiguous 8KB/partition).
  2. Fused [Wk|Wq] projection: pkq[128,512] += wkq[:,c8]^T @ xn[:,c8] over c8;
     rows 0:64 = k^T, rows 64:128 = q^T. v^T likewise into pv[64,512].
     Copy pkq -> kqT fp16 (k^T serves as scores stationary from partitions
     0:64; q^T is the moving operand read at partition offset 64:128 - no
     shift DMA). v^T -> PE-transpose -> v_aug[s,65] = [v | 1] (the ones
     column makes the PV matmul also produce the softmax denominator).
  3. Attention in scoresT layout (keys on partitions), scores emitted two
     chunks ahead of PV so the PE never waits for ACT's exp:
       scoresT psum = kT-chunk.T @ qT-block      (PE)
       probsT = exp(scoresT / 32) -> fp16        (ACT, PSUM->SBUF)
       diagonal chunk: probsT *= causal mask     (DVE, all-SBUF fp16 4x mode)
       outT_aug[65,512] += v_aug-chunk.T @ probsT (PE, PSUM accumulation)
  4. Epilogue per q-block: PE-transpose outT_aug -> [t,65]; multiply by the
     reciprocal of the denominator column; store out as [128, t-tile, 64]
     (contiguous 4KB/partition; host untransposes).
Cross-iteration software pipeline: the For_i body holds two ping-pong virtual
iterations (double-buffered kqT/v_aug/out_sb), so the final attention block of
iteration i overlaps iteration i+1's projections and x DMA.
"""

import numpy as np

B, T, C, D = 8, 2048, 1024, 64
NT = T // 128       # 16 t-tiles
NC8 = C // 128      # 8 c-chunks
QB = T // 512       # 4 q-blocks
SCALE = 1.0 / np.sqrt(C)

_CACHE = {}


def build_nc(reps: int = 1):
    import concourse.tile as tile
    import concourse.bass as bass
    from concourse import bacc, mybir
    from concourse.masks import make_identity

    f32 = mybir.dt.float32
    f16 = mybir.dt.float16

    nc = bacc.Bacc("TRN2", target_bir_lowering=False, debug=False)
    xTb = nc.dram_tensor("xTb", [128, QB, NC8, 512], f16, kind="ExternalInput").ap()
    wkq_d = nc.dram_tensor("wkq", [128, NC8, 128], f16, kind="ExternalInput").ap()
    wv_d = nc.dram_tensor("wv", [128, NC8, D], f16, kind="ExternalInput").ap()
    tri_d = nc.dram_tensor("tri", [128, 128], f16, kind="ExternalInput").ap()
    out_d = nc.dram_tensor("out", [128, NT, D], f32, kind="ExternalOutput").ap()

    with tile.TileContext(nc) as tc:
        with (
            tc.tile_pool(name="const", bufs=1) as constp,
            tc.tile_pool(name="persist", bufs=1) as persist,
            tc.tile_pool(name="xn", bufs=3) as xnp,
            tc.tile_pool(name="vt", bufs=2) as vtp,
            tc.tile_pool(name="probs", bufs=5) as probsp,
            tc.tile_pool(name="oT", bufs=2) as oTp,
            tc.tile_pool(name="qk_ps", bufs=1, space="PSUM") as qk_ps,
            tc.tile_pool(name="v_ps", bufs=1, space="PSUM") as v_ps,
            tc.tile_pool(name="sc_ps", bufs=3, space="PSUM") as sc_ps,
            tc.tile_pool(name="o_ps", bufs=2, space="PSUM") as o_ps,
            tc.tile_pool(name="tp_ps", bufs=1, space="PSUM") as tp_ps,
        ):
            ident = constp.tile([128, 128], f32)
            make_identity(nc, ident[:])
            identh = constp.tile([128, 128], f16)
            nc.vector.tensor_copy(identh[:], ident[:])
            tri = constp.tile([128, 128], f16)
            nc.sync.dma_start(tri[:], tri_d)
            wkq = constp.tile([128, NC8, 128], f16)
            nc.sync.dma_start(wkq[:], wkq_d)
            wv = constp.tile([128, NC8, D], f16)
            nc.sync.dma_start(wv[:], wv_d)

            kqT = [persist.tile([128, T], f16, name=f"kqT{b}") for b in (0, 1)]
            qTs = [persist.tile([64, T], f16, name=f"qTs{b}") for b in (0, 1)]
            vaug = [persist.tile([128, NT, D + 1], f16, name=f"vaug{b}")
                    for b in (0, 1)]
            outsb = [persist.tile([128, NT, D], f32, name=f"outsb{b}")
                     for b in (0, 1)]
            rcp4 = persist.tile([128, 4], f32)
            for b in (0, 1):
                nc.vector.memset(vaug[b][:, :, D], 1.0)

            def front_ops(nb, b):
                """Closures: x DMA, 8 fused projection mm pairs, tail."""
                st = {}

                def load():
                    xn = xnp.tile([128, NC8, 512], f16, tag="xn",
                                  name=f"xn{nb}_{b}")
                    nc.sync.dma_start(xn[:], xTb[:, nb])
                    st["xn"] = xn
                    st["pkq"] = qk_ps.tile([128, 512], f32, tag="pkq", name=f"pkq{nb}_{b}")
                    st["pv"] = v_ps.tile([64, 512], f32, tag="pv", name=f"pv{nb}_{b}")

                def mm(c8):
                    def f():
                        nc.tensor.matmul(st["pkq"][:], wkq[:, c8], st["xn"][:, c8],
                                         start=(c8 == 0), stop=(c8 == NC8 - 1))
                        nc.tensor.matmul(st["pv"][:], wv[:, c8], st["xn"][:, c8],
                                         start=(c8 == 0), stop=(c8 == NC8 - 1))
                    return f

                def tail_kq():
                    nb_s = slice(512 * nb, 512 * (nb + 1))
                    nc.vector.tensor_copy(kqT[b][:, nb_s], st["pkq"][:])
                    nc.scalar.dma_start(qTs[b][:, nb_s], kqT[b][64:128, nb_s])

                def tail_v():
                    vt = vtp.tile([64, 512], f16, tag="vt", name=f"vt{nb}_{b}")
                    nc.vector.tensor_copy(vt[:], st["pv"][:])
                    tpv = tp_ps.tile([128, 4, D], f16, tag="tp",
                                     name=f"tpv{nb}_{b}")
                    for i in range(4):
                        nc.tensor.transpose(tpv[:, i], vt[:, 128 * i:128 * (i + 1)],
                                            identh[0:64, 0:64])
                    nc.vector.tensor_copy(vaug[b][:, 4 * nb:4 * nb + 4, 0:D],
                                          tpv[:])

                return [load] + [mm(c) for c in range(NC8)] + [tail_kq, tail_v]

            def attn_ops(qb, b):
                """Scores emitted 2 chunks ahead of PV, then epilogue."""
                nsc = 4 * (qb + 1)
                st = {}

                def lo_of(c):
                    j = c - 4 * qb
                    return 128 * j if j > 0 else 0

                def score(c):
                    def f():
                        if c == 0:
                            st["po"] = o_ps.tile([D + 1, 512], f32, tag="po",
                                                 name=f"po{qb}_{b}")
                        lo = lo_of(c)
                        j = c - 4 * qb
                        psc = sc_ps.tile([128, 512], f32, tag="psc", name=f"psc{qb}_{b}_{c}")
                        nc.tensor.matmul(
                            psc[:, lo:512], kqT[b][0:64, 128 * c:128 * (c + 1)],
                            qTs[b][:, 512 * qb + lo:512 * (qb + 1)],
                            start=True, stop=True)
                        probs = probsp.tile([128, 512], f16, tag="probs", name=f"probs{qb}_{b}_{c}")
                        nc.scalar.activation(probs[:, lo:512], psc[:, lo:512],
                                             mybir.ActivationFunctionType.Exp,
                                             scale=float(SCALE))
                        if j >= 0:
                            nc.vector.tensor_mul(probs[:, lo:lo + 128],
                                                 probs[:, lo:lo + 128], tri[:])
                        st[c] = probs
                    return f

                def pv(c):
                    def f():
                        lo = lo_of(c)
                        nc.tensor.matmul(st["po"][:, lo:512], vaug[b][:, c, :],
                                         st.pop(c)[:, lo:512],
                                         start=(c == 0), stop=(c == nsc - 1))
                    return f

                def epilogue():
                    oT = oTp.tile([D + 1, 512], f16, tag="oT", name=f"oT{qb}_{b}")
                    nc.vector.tensor_copy(oT[:], st["po"][:])
                    for j in range(4):
                        pt = sc_ps.tile([128, D + 1], f16, tag="psc",
                                        name=f"otr{qb}_{b}_{j}")
                        nc.tensor.transpose(pt[:], oT[:, 128 * j:128 * (j + 1)],
                                            identh[0:D + 1, 0:D + 1])
                        nc.vector.reciprocal(rcp4[:, j:j + 1], pt[:, D:D + 1])
                        nc.vector.tensor_scalar_mul(outsb[b][:, 4 * qb + j, :],
                                                    pt[:, 0:D], rcp4[:, j:j + 1])
                    if qb == QB - 1:
                        nc.sync.dma_start(out_d, outsb[b][:])

                # scores run 2 chunks ahead of the matching PV accumulation
                ops = [score(0)]
                for c in range(nsc):
                    if c + 1 < nsc:
                        ops.append(score(c + 1))
                    ops.append(pv(c))
                return ops + [epilogue]

            def interleave(a, pend):
                """Merge op streams evenly."""
                if not pend:
                    return a
                out = []
                ratio = len(a) / len(pend)
                ai = 0
                for bi, bop in enumerate(pend):
                    target = int(round((bi + 1) * ratio))
                    out.extend(a[ai:target])
                    ai = target
                    out.append(bop)
                out.extend(a[ai:])
                return out

            def half(b, pending):
                """Emit one virtual iteration's fronts for buffer b,
                consuming `pending` plus this iteration's attn 0..2;
                return the attn(3) closures left pending."""
                for nb in range(QB):
                    for op in interleave(front_ops(nb, b), pending):
                        op()
                    pending = attn_ops(nb, b)
                return pending

            if reps == 1:
                pending = half(0, [])
            else:
                assert reps % 4 == 1, "pipelined build needs reps % 4 == 1"
                pending = half(0, [])
                from concourse import mybir as _mb
                with tc.For_i(0, (reps - 1) // 4, 1, hint_engines=(
                        _mb.EngineType.PE, _mb.EngineType.Activation,
                        _mb.EngineType.DVE, _mb.EngineType.SP,
                        _mb.EngineType.Pool)):
                    pending = half(1, pending)
                    pending = half(0, pending)
                    pending = half(1, pending)
                    pending = half(0, pending)
            for op in pending:
                op()
    nc.compile()
    return nc


def prep_inputs(x, Wq, Wk, Wv):
    """Host marshaling into the on-chip layout: per-core input dicts."""
    x = np.asarray(x, dtype=np.float32)
    Wq = np.asarray(Wq, dtype=np.float32)
    Wk = np.asarray(Wk, dtype=np.float32)
    Wv = np.asarray(Wv, dtype=np.float32)
    # xTb[b][p, nb, c8, t'] = x[b, nb*512 + t', c8*128 + p]
    xTb = x.reshape(B, QB, 512, NC8, 128).transpose(0, 4, 1, 3, 2)
    xTb = np.ascontiguousarray(xTb).astype(np.float16)
    wkq = np.empty((128, NC8, 128), dtype=np.float16)
    wkq[:, :, 0:64] = Wk.reshape(NC8, 128, D).transpose(1, 0, 2)
    wkq[:, :, 64:128] = Wq.reshape(NC8, 128, D).transpose(1, 0, 2)
    wv = np.ascontiguousarray(
        Wv.reshape(NC8, 128, D).transpose(1, 0, 2)).astype(np.float16)
    tri = np.triu(np.ones((128, 128), dtype=np.float16))
    return [{"xTb": xTb[b], "wkq": wkq, "wv": wv, "tri": tri}
            for b in range(B)]


def unshuffle_out(res):
    """[128, NT, D] per-core tile layout -> [T, D]."""
    return np.ascontiguousarray(
        np.asarray(res).transpose(1, 0, 2).reshape(T, D))


class _SpmdRunner:
    """Builds the jitted sharded callable once; reusable across calls."""

    def __init__(self, nc, n_cores=8):
        import jax
        import jax.numpy as jnp
        from jax.sharding import Mesh, PartitionSpec
        from jax.experimental.shard_map import shard_map
        from concourse import mybir
        from concourse.bass2jax import (_bass_exec_p, install_neuronx_cc_hook,
                                        partition_id_tensor)

        install_neuronx_cc_hook()
        self.jax = jax
        self.jnp = jnp
        self.n_cores = n_cores
        partition_name = (nc.partition_id_tensor.name
                          if nc.partition_id_tensor else None)
        in_names, out_names, out_avals, zero_outs = [], [], [], []
        for alloc in nc.m.functions[0].allocations:
            if not isinstance(alloc, mybir.MemoryLocationSet):
                continue
            name = alloc.memorylocations[0].name
            if alloc.kind == "ExternalInput":
                if name != partition_name:
                    in_names.append(name)
            elif alloc.kind == "ExternalOutput":
                out_names.append(name)
                shape = tuple(alloc.tensor_shape)
                dtype = mybir.dt.np(alloc.dtype)
                out_avals.append(jax.core.ShapedArray(shape, dtype))
                zero_outs.append((shape, dtype))
        self.in_names, self.out_names = in_names, out_names
        self.out_avals, self.zero_outs = out_avals, zero_outs
        n_params = len(in_names)
        self.n_params = n_params
        all_in_names = list(in_names) + list(out_names)
        if partition_name is not None:
            all_in_names.append(partition_name)

        def _body(*args):
            operands = list(args)
            if partition_name is not None:
                operands.append(partition_id_tensor())
            outs = _bass_exec_p.bind(
                *operands,
                out_avals=tuple(out_avals),
                in_names=tuple(all_in_names),
                out_names=tuple(out_names),
                lowering_input_output_aliases=(),
                sim_require_finite=True,
                sim_require_nnan=True,
                nc=nc,
            )
            return tuple(outs)

        devices = jax.devices()[:n_cores]
        mesh = Mesh(np.asarray(devices), ("core",))
        n_outs = len(out_names)
        in_specs = (PartitionSpec("core"),) * (n_params + n_outs)
        out_specs = (PartitionSpec("core"),) * n_outs
        donate = tuple(range(n_params, n_params + n_outs))
        self.sharded = jax.jit(
            shard_map(_body, mesh=mesh, in_specs=in_specs,
                      out_specs=out_specs, check_rep=False),
            donate_argnums=donate, keep_unused=True)
        self._zeros_fn = jax.jit(
            lambda: tuple(jnp.zeros((n_cores * s[0], *s[1:]), d)
                          for (s, d) in zero_outs))

    def put_inputs(self, in_maps):
        per_core = [[np.asarray(m[n]) for n in self.in_names] for m in in_maps]
        concat = [np.concatenate([per_core[c][i] for c in range(self.n_cores)], axis=0)
                  for i in range(self.n_params)]
        return [self.jax.device_put(a) for a in concat]

    def make_zeros_dev(self):
        z = self._zeros_fn()
        self.jax.block_until_ready(z)
        return list(z)

    def run(self, dev_in, zeros=None):
        if zeros is None:
            zeros = self.make_zeros_dev()
        outs = self.sharded(*dev_in, *zeros)
        self.jax.block_until_ready(outs)
        return outs

    def gather(self, outs):
        return [
            {n: np.asarray(outs[i]).reshape(self.n_cores, *self.out_avals[i].shape)[c]
             for i, n in enumerate(self.out_names)}
            for c in range(self.n_cores)
        ]


def _get_runner():
    if "runner" not in _CACHE:
        _CACHE["runner"] = _SpmdRunner(build_nc(reps=1), n_cores=B)
    return _CACHE["runner"]


def kernel(x, Wq, Wk, Wv):
    runner = _get_runner()
    in_maps = prep_inputs(x, Wq, Wk, Wv)
    dev_in = runner.put_inputs(in_maps)
    res = runner.gather(runner.run(dev_in))
    return np.stack([unshuffle_out(res[b]["out"]) for b in range(B)], axis=0)


# revision 17
# speedup vs baseline: 1.4766x; 1.4766x over previous
"""Trainium2 Bass kernel for single-head causal attention.

Problem: nn_Attention (dense_transformer): B=8, T=2048, C=1024, D=64, fp32.
    q = x @ Wq; k = x @ Wk; v = x @ Wv
    out = softmax(causal(q k^T / sqrt(C))) @ v

Sharding: data-parallel over batch - one batch element per NeuronCore (8 cores).
Weights replicated. The host marshals inputs into the on-chip layout (fp16,
x pre-transposed to [C, T] and blocked per DMA tile); each core runs an
identical program on its batch element.

Per-core algorithm (fp16 operands, fp32 PSUM accumulation):
  1. x^T arrives via DMA as xn[128, c8, 512] blocks (contiguous 8KB/partition).
  2. Fused [Wk|Wq] projection: pkq[128,512] += wkq[:,c8]^T @ xn[:,c8] over c8;
     rows 0:64 = k^T, rows 64:128 = q^T. v^T likewise into pv[64,512].
     Copy pkq -> kqT fp16 (k^T serves as scores stationary from partitions
     0:64; q^T is the moving operand read at partition offset 64:128 - no
     shift DMA). v^T -> PE-transpose -> v_aug[s,65] = [v | 1] (the ones
     column makes the PV matmul also produce the softmax denominator).
  3. Attention in scoresT layout (keys on partitions), scores emitted two
     chunks ahead of PV so the PE never waits for ACT's exp:
       scoresT psum = kT-chunk.T @ qT-block      (PE)
       probsT = exp(scoresT / 32) -> fp16        (ACT, PSUM->SBUF)
       diagonal chunk: probsT *= causal mask     (DVE, all-SBUF fp16 4x mode)
       outT_aug[65,512] += v_aug-chunk.T @ probsT (PE, PSUM accumulation)
  4. Epilogue per q-block: PE-transpose outT_aug -> [t,65]; multiply by the
     reciprocal of the denominator column; store out as [128, t-tile, 64]
     (contiguous 4KB/partition; host untransposes).
Cross-iteration software pipeline: the For_i body holds two ping-pong virtual
iterations (double-buffered kqT/v_aug/out_sb), so the final attention block of
iteration i overlaps iteration i+1's projections and x DMA.
"""

import numpy as np

B, T, C, D = 8, 2048, 1024, 64
NT = T // 128       # 16 t-tiles
NC8 = C // 128      # 8 c-chunks
QB = T // 512       # 4 q-blocks
SCALE = 1.0 / np.sqrt(C)

_CACHE = {}
PHASE = 2
QK_BUFS, V_BUFS, SC_BUFS, O_BUFS = 1, 1, 5, 1
AHEAD = 2
PROBS_BUFS = 5


def build_nc(reps: int = 1):
    import concourse.tile as tile
    import concourse.bass as bass
    from concourse import bacc, mybir
    from concourse.masks import make_identity

    f32 = mybir.dt.float32
    f16 = mybir.dt.float16

    nc = bacc.Bacc("TRN2", target_bir_lowering=False, debug=False)
    xTb = nc.dram_tensor("xTb", [128, QB, NC8, 512], f16, kind="ExternalInput").ap()
    wkq_d = nc.dram_tensor("wkq", [128, NC8, 128], f16, kind="ExternalInput").ap()
    wv_d = nc.dram_tensor("wv", [128, NC8, D], f16, kind="ExternalInput").ap()
    tri_d = nc.dram_tensor("tri", [128, 128], f16, kind="ExternalInput").ap()
    out_d = nc.dram_tensor("out", [128, NT, D], f32, kind="ExternalOutput").ap()

    with tile.TileContext(nc) as tc:
        with (
            tc.tile_pool(name="const", bufs=1) as constp,
            tc.tile_pool(name="persist", bufs=1) as persist,
            tc.tile_pool(name="xn", bufs=3) as xnp,
            tc.tile_pool(name="vt", bufs=2) as vtp,
            tc.tile_pool(name="probs", bufs=PROBS_BUFS) as probsp,
            tc.tile_pool(name="oT", bufs=2) as oTp,
            tc.tile_pool(name="qk_ps", bufs=QK_BUFS, space="PSUM") as qk_ps,
            tc.tile_pool(name="v_ps", bufs=V_BUFS, space="PSUM") as v_ps,
            tc.tile_pool(name="sc_ps", bufs=SC_BUFS, space="PSUM") as sc_ps,
            tc.tile_pool(name="o_ps", bufs=O_BUFS, space="PSUM") as o_ps,
        ):
            ident = constp.tile([128, 128], f32)
            make_identity(nc, ident[:])
            identh = constp.tile([128, 128], f16)
            nc.vector.tensor_copy(identh[:], ident[:])
            tri = constp.tile([128, 128], f16)
            nc.sync.dma_start(tri[:], tri_d)
            wkq = constp.tile([128, NC8, 128], f16)
            nc.sync.dma_start(wkq[:], wkq_d)
            wv = constp.tile([128, NC8, D], f16)
            nc.sync.dma_start(wv[:], wv_d)

            kqT = [persist.tile([128, T], f16, name=f"kqT{b}") for b in (0, 1)]
            qTs = [persist.tile([64, T], f16, name=f"qTs{b}") for b in (0, 1)]
            vaug = [persist.tile([128, NT, D + 1], f16, name=f"vaug{b}")
                    for b in (0, 1)]
            outsb = [persist.tile([128, NT, D], f32, name=f"outsb{b}")
                     for b in (0, 1)]
            rcp4 = persist.tile([128, 4], f32)
            for b in (0, 1):
                nc.vector.memset(vaug[b][:, :, D], 1.0)

            def front_ops(nb, b):
                """Closures: x DMA, 8 fused projection mm pairs, tail."""
                st = {}

                def load():
                    xn = xnp.tile([128, NC8, 512], f16, tag="xn",
                                  name=f"xn{nb}_{b}")
                    nc.sync.dma_start(xn[:], xTb[:, nb])
                    st["xn"] = xn
                    st["pkq"] = qk_ps.tile([128, 512], f32, tag="pkq", name=f"pkq{nb}_{b}")
                    st["pv"] = v_ps.tile([64, 512], f32, tag="pv", name=f"pv{nb}_{b}")

                def mm_kq(c8):
                    def f():
                        nc.tensor.matmul(st["pkq"][:], wkq[:, c8], st["xn"][:, c8],
                                         start=(c8 == 0), stop=(c8 == NC8 - 1))
                    return f

                def mm_v(c8):
                    def f():
                        nc.tensor.matmul(st["pv"][:], wv[:, c8], st["xn"][:, c8],
                                         start=(c8 == 0), stop=(c8 == NC8 - 1))
                    return f

                def tail_kq():
                    nb_s = slice(512 * nb, 512 * (nb + 1))
                    nc.vector.tensor_copy(kqT[b][:, nb_s], st["pkq"][:])
                    nc.scalar.dma_start(qTs[b][:, nb_s], kqT[b][64:128, nb_s])

                def tail_v():
                    vt = vtp.tile([64, 512], f16, tag="vt", name=f"vt{nb}_{b}")
                    nc.vector.tensor_copy(vt[:], st["pv"][:])
                    tpv = sc_ps.tile([128, 4, D], f16, tag="psc",
                                     name=f"tpv{nb}_{b}")
                    for i in range(4):
                        nc.tensor.transpose(tpv[:, i], vt[:, 128 * i:128 * (i + 1)],
                                            identh[0:64, 0:64])
                    nc.vector.tensor_copy(vaug[b][:, 4 * nb:4 * nb + 4, 0:D],
                                          tpv[:])

                return ([load] + [mm_kq(c) for c in range(NC8)] + [tail_kq]
                        + [mm_v(c) for c in range(NC8)] + [tail_v])

            def attn_ops(qb, b):
                """Scores emitted 2 chunks ahead of PV, then epilogue."""
                nsc = 4 * (qb + 1)
                st = {}

                def lo_of(c):
                    j = c - 4 * qb
                    return 128 * j if j > 0 else 0

                def score(c):
                    def f():
                        if c == 0:
                            st["po"] = o_ps.tile([D + 1, 512], f32, tag="po",
                                                 name=f"po{qb}_{b}")
                        lo = lo_of(c)
                        j = c - 4 * qb
                        psc = sc_ps.tile([128, 512], f32, tag="psc", name=f"psc{qb}_{b}_{c}")
                        nc.tensor.matmul(
                            psc[:, lo:512], kqT[b][0:64, 128 * c:128 * (c + 1)],
                            qTs[b][:, 512 * qb + lo:512 * (qb + 1)],
                            start=True, stop=True)
                        probs = probsp.tile([128, 512], f16, tag="probs", name=f"probs{qb}_{b}_{c}")
                        nc.scalar.activation(probs[:, lo:512], psc[:, lo:512],
                                             mybir.ActivationFunctionType.Exp,
                                             scale=float(SCALE))
                        if j >= 0:
                            nc.gpsimd.tensor_mul(probs[:, lo:lo + 128],
                                                 probs[:, lo:lo + 128], tri[:])
                        st[c] = probs
                    return f

                def pv(c):
                    def f():
                        lo = lo_of(c)
                        nc.tensor.matmul(st["po"][:, lo:512], vaug[b][:, c, :],
                                         st.pop(c)[:, lo:512],
                                         start=(c == 0), stop=(c == nsc - 1))
                    return f

                def epilogue():
                    oT = oTp.tile([D + 1, 512], f16, tag="oT", name=f"oT{qb}_{b}")
                    nc.vector.tensor_copy(oT[:], st["po"][:])
                    for j in range(4):
                        pt = sc_ps.tile([128, D + 1], f16, tag="psc",
                                        name=f"otr{qb}_{b}_{j}")
                        nc.tensor.transpose(pt[:], oT[:, 128 * j:128 * (j + 1)],
                                            identh[0:D + 1, 0:D + 1])
                        nc.vector.reciprocal(rcp4[:, j:j + 1], pt[:, D:D + 1])
                        nc.vector.tensor_scalar_mul(outsb[b][:, 4 * qb + j, :],
                                                    pt[:, 0:D], rcp4[:, j:j + 1])
                    if qb == QB - 1:
                        nc.sync.dma_start(out_d, outsb[b][:])

                # scores run AHEAD chunks ahead of the matching PV
                ops = [score(c) for c in range(min(AHEAD, nsc))]
                for c in range(nsc):
                    if c + AHEAD < nsc:
                        ops.append(score(c + AHEAD))
                    ops.append(pv(c))
                return ops + [epilogue]

            def interleave(a, pend, phase=PHASE):
                """Merge op streams evenly; first `phase` a-ops lead."""
                if not pend:
                    return a
                out = list(a[:phase])
                rest = a[phase:]
                ratio = len(rest) / len(pend)
                ai = 0
                for bi, bop in enumerate(pend):
                    target = int(round((bi + 1) * ratio))
                    out.extend(rest[ai:target])
                    ai = target
                    out.append(bop)
                out.extend(rest[ai:])
                return out

            def half(b, pending):
                """Emit one virtual iteration's fronts for buffer b,
                consuming `pending` plus this iteration's attn 0..2;
                return the attn(3) closures left pending."""
                for nb in range(QB):
                    for op in interleave(front_ops(nb, b), pending):
                        op()
                    pending = attn_ops(nb, b)
                return pending

            if reps == 1:
                pending = half(0, [])
            else:
                assert reps % 8 == 1, "pipelined build needs reps % 8 == 1"
                pending = half(0, [])
                from concourse import mybir as _mb
                with tc.For_i(0, (reps - 1) // 8, 1, hint_engines=(
                        _mb.EngineType.PE, _mb.EngineType.Activation,
                        _mb.EngineType.DVE, _mb.EngineType.SP,
                        _mb.EngineType.Pool)):
                    for _u in range(4):
                        pending = half(1, pending)
                        pending = half(0, pending)
            for op in pending:
                op()
    nc.compile()
    return nc


def prep_inputs(x, Wq, Wk, Wv):
    """Host marshaling into the on-chip layout: per-core input dicts."""
    x = np.asarray(x, dtype=np.float32)
    Wq = np.asarray(Wq, dtype=np.float32)
    Wk = np.asarray(Wk, dtype=np.float32)
    Wv = np.asarray(Wv, dtype=np.float32)
    # xTb[b][p, nb, c8, t'] = x[b, nb*512 + t', c8*128 + p]
    xTb = x.reshape(B, QB, 512, NC8, 128).transpose(0, 4, 1, 3, 2)
    xTb = np.ascontiguousarray(xTb).astype(np.float16)
    wkq = np.empty((128, NC8, 128), dtype=np.float16)
    wkq[:, :, 0:64] = Wk.reshape(NC8, 128, D).transpose(1, 0, 2)
    wkq[:, :, 64:128] = Wq.reshape(NC8, 128, D).transpose(1, 0, 2)
    wv = np.ascontiguousarray(
        Wv.reshape(NC8, 128, D).transpose(1, 0, 2)).astype(np.float16)
    tri = np.triu(np.ones((128, 128), dtype=np.float16))
    return [{"xTb": xTb[b], "wkq": wkq, "wv": wv, "tri": tri}
            for b in range(B)]


def unshuffle_out(res):
    """[128, NT, D] per-core tile layout -> [T, D]."""
    return np.ascontiguousarray(
        np.asarray(res).transpose(1, 0, 2).reshape(T, D))


class _SpmdRunner:
    """Builds the jitted sharded callable once; reusable across calls."""

    def __init__(self, nc, n_cores=8):
        import jax
        import jax.numpy as jnp
        from jax.sharding import Mesh, PartitionSpec
        from jax.experimental.shard_map import shard_map
        from concourse import mybir
        from concourse.bass2jax import (_bass_exec_p, install_neuronx_cc_hook,
                                        partition_id_tensor)

        install_neuronx_cc_hook()
        self.jax = jax
        self.jnp = jnp
        self.n_cores = n_cores
        partition_name = (nc.partition_id_tensor.name
                          if nc.partition_id_tensor else None)
        in_names, out_names, out_avals, zero_outs = [], [], [], []
        for alloc in nc.m.functions[0].allocations:
            if not isinstance(alloc, mybir.MemoryLocationSet):
                continue
            name = alloc.memorylocations[0].name
            if alloc.kind == "ExternalInput":
                if name != partition_name:
                    in_names.append(name)
            elif alloc.kind == "ExternalOutput":
                out_names.append(name)
                shape = tuple(alloc.tensor_shape)
                dtype = mybir.dt.np(alloc.dtype)
                out_avals.append(jax.core.ShapedArray(shape, dtype))
                zero_outs.append((shape, dtype))
        self.in_names, self.out_names = in_names, out_names
        self.out_avals, self.zero_outs = out_avals, zero_outs
        n_params = len(in_names)
        self.n_params = n_params
        all_in_names = list(in_names) + list(out_names)
        if partition_name is not None:
            all_in_names.append(partition_name)

        def _body(*args):
            operands = list(args)
            if partition_name is not None:
                operands.append(partition_id_tensor())
            outs = _bass_exec_p.bind(
                *operands,
                out_avals=tuple(out_avals),
                in_names=tuple(all_in_names),
                out_names=tuple(out_names),
                lowering_input_output_aliases=(),
                sim_require_finite=True,
                sim_require_nnan=True,
                nc=nc,
            )
            return tuple(outs)

        devices = jax.devices()[:n_cores]
        mesh = Mesh(np.asarray(devices), ("core",))
        n_outs = len(out_names)
        in_specs = (PartitionSpec("core"),) * (n_params + n_outs)
        out_specs = (PartitionSpec("core"),) * n_outs
        donate = tuple(range(n_params, n_params + n_outs))
        self.sharded = jax.jit(
            shard_map(_body, mesh=mesh, in_specs=in_specs,
                      out_specs=out_specs, check_rep=False),
            donate_argnums=donate, keep_unused=True)
        self._zeros_fn = jax.jit(
            lambda: tuple(jnp.zeros((n_cores * s[0], *s[1:]), d)
                          for (s, d) in zero_outs))

    def put_inputs(self, in_maps):
        per_core = [[np.asarray(m[n]) for n in self.in_names] for m in in_maps]
        concat = [np.concatenate([per_core[c][i] for c in range(self.n_cores)], axis=0)
                  for i in range(self.n_params)]
        return [self.jax.device_put(a) for a in concat]

    def make_zeros_dev(self):
        z = self._zeros_fn()
        self.jax.block_until_ready(z)
        return list(z)

    def run(self, dev_in, zeros=None):
        if zeros is None:
            zeros = self.make_zeros_dev()
        outs = self.sharded(*dev_in, *zeros)
        self.jax.block_until_ready(outs)
        return outs

    def gather(self, outs):
        return [
            {n: np.asarray(outs[i]).reshape(self.n_cores, *self.out_avals[i].shape)[c]
             for i, n in enumerate(self.out_names)}
            for c in range(self.n_cores)
        ]


def _get_runner():
    if "runner" not in _CACHE:
        _CACHE["runner"] = _SpmdRunner(build_nc(reps=1), n_cores=B)
    return _CACHE["runner"]


def kernel(x, Wq, Wk, Wv):
    runner = _get_runner()
    in_maps = prep_inputs(x, Wq, Wk, Wv)
    dev_in = runner.put_inputs(in_maps)
    res = runner.gather(runner.run(dev_in))
    return np.stack([unshuffle_out(res[b]["out"]) for b in range(B)], axis=0)
